# revision 19
# baseline (speedup 1.0000x reference)
"""Trainium2 Bass kernel for nn_DiagRNN (diagonal complex linear RNN / LRU).

  y = Re[C @ h] + D*x,  h_t = A h_{t-1} + B x_t  (A complex-diagonal)

Strategy (8 NeuronCores, sequence-parallel):
  * Sequence of L=16384 split into 32 chunks of T=512. Chunk m is processed
    by core m%8 in "slot" m//8 (interleaved assignment) so cross-core carry
    exchange is a small per-slot AllGather that pipelines behind compute.
  * Complex scan is reduced to two REAL first-order scans per chunk via a
    rotating-frame transform: with A = r*e^{i\theta},
        W_k = e^{-i\theta k} h_{mT+k}  satisfies  W_k = r W_{k-1} + g_k,
        g_k = e^{-i\theta k} (B x)_{mT+k}.
    The real scans run on the DVE hardware scan instruction
    (tensor_tensor_scan).
  * All per-slice elementwise work stays on the DVE (GpSimd tensor ops
    measured ~3x slower and contend with the DVE for SBUF ports).  Act does
    the PSUM->SBUF pb copies, the W-last column publishes and the
    carry-correction scaled-table copies.
  * Carries (direct form, no serial recursion): cores publish raw W-last
    [128,16] f16 (re/im interleaved), AllGather them, and each consumer
    DMA-copies the gathered block for slot s into a persistent history tile
    eg_all[:, 128s:128s+128].  The incoming state vp for a chunk is ONE
    weighted reduction over the whole history:
        vp[h] = sum_{s',j} G_{s-s'}(j,h) * Wlast_{s',j}(h)
    with host-precomputed complex fold tables G (ROT1/ROTT1 folded in),
    stored as interleaved re/im f16 tables fre/fim [128, 4*512].
    Per consume: 2 DVE tensor_tensor + 4 reduces -> vre/vim [128,8].
  * The correction enters the OUTPUT u directly:
        u = (cos.Wre - sin.Wim) + r^{k+1}(cos.vp_re - sin.vp_im)
    using pre-multiplied tables cpw=cos*r^{k+1}, spw=-sin*r^{k+1} and the
    Act engine's per-partition scale: ucorr = Copy(table, scale=vp).
  * D*x is folded into the C matmul PSUM group as a diag(D) stationary.
  * Matmuls (B_re, B_im, C, diagD) run on the PE with fp16 operands;
    accumulation is fp32 in PSUM.  psy matmuls are emitted after scan
    slice 4 of the following slot so the PE never head-blocks on u-tiles.
"""
import sys, os
sys.path.insert(0, '/opt/trn_rl_repo')
import numpy as np

import concourse.bass as bass
import concourse.bacc as bacc
import concourse.tile as tile
import concourse.mybir as mybir
from concourse.bass_utils import run_bass_kernel_spmd

L, H, M = 16384, 1024, 1024
NC = 8
T = 512
S = L // (T * NC)          # 4 slots
NSL = H // 128             # 8 slices

f32 = mybir.dt.float32
f16 = mybir.dt.float16
AL = mybir.AluOpType
AX = mybir.AxisListType
AF = mybir.ActivationFunctionType

TABLE_DT = f16
MM_DT = f16

_BUILD_CACHE = {}


def _build():
    if "nc" in _BUILD_CACHE:
        return _BUILD_CACHE["nc"]
    nc = bacc.Bacc("TRN2", target_bir_lowering=False, debug=False,
                   num_devices=NC)

    xt_d = nc.dram_tensor("xt", [S, M, T], MM_DT, kind="ExternalInput").ap()
    brt_d = nc.dram_tensor("brt", [M, H], MM_DT, kind="ExternalInput").ap()
    bit_d = nc.dram_tensor("bit", [M, H], MM_DT, kind="ExternalInput").ap()
    ct_d = nc.dram_tensor("ct", [H, M], MM_DT, kind="ExternalInput").ap()
    dd_d = nc.dram_tensor("ddiag", [128, H], MM_DT, kind="ExternalInput").ap()
    cos_d = nc.dram_tensor("costb", [H, T], TABLE_DT, kind="ExternalInput").ap()
    sin_d = nc.dram_tensor("sintb", [H, T], TABLE_DT, kind="ExternalInput").ap()
    ns_d = nc.dram_tensor("nstb", [H, T], TABLE_DT, kind="ExternalInput").ap()
    cpw_d = nc.dram_tensor("cpwtb", [H, T], TABLE_DT, kind="ExternalInput").ap()
    spw_d = nc.dram_tensor("spwtb", [H, T], TABLE_DT, kind="ExternalInput").ap()
    fre_d = nc.dram_tensor("fretb", [128, S * S * 128], TABLE_DT, kind="ExternalInput").ap()
    fim_d = nc.dram_tensor("fimtb", [128, S * S * 128], TABLE_DT, kind="ExternalInput").ap()
    rcol_d = nc.dram_tensor("rcol", [128, 8], f32, kind="ExternalInput").ap()
    y_d = nc.dram_tensor("y", [S, M, T], f16, kind="ExternalOutput").ap()

    with tile.TileContext(nc) as tc:
        with tc.tile_pool(name="pw", bufs=1) as pw, \
             tc.tile_pool(name="px", bufs=1) as px, \
             tc.tile_pool(name="pg", bufs=1) as pg, \
             tc.tile_pool(name="pc", bufs=1) as pcp, \
             tc.tile_pool(name="pp", bufs=1, space="PSUM") as pp, \
             tc.tile_pool(name="pd", bufs=1, space="DRAM") as pd:

            # ---------- persistent weights / tables ----------
            brt_sb = []
            bit_sb = []
            ct_sb = []
            cos_sb = []
            sin_sb = []
            ns_sb = []
            cpw_sb = []
            spw_sb = []
            brtB_sb = []
            bitB_sb = []
            for d in range(NSL):
                brt_sb.append(pw.tile([128, H // 2], MM_DT, name=f"brtA{d}"))
                brtB_sb.append(pw.tile([128, H // 2], MM_DT, name=f"brtB{d}"))
                bit_sb.append(pw.tile([128, H // 2], MM_DT, name=f"bitA{d}"))
                bitB_sb.append(pw.tile([128, H // 2], MM_DT, name=f"bitB{d}"))
                ct_sb.append(pw.tile([128, M], MM_DT, name=f"ct{d}"))
                cos_sb.append(pw.tile([128, T], TABLE_DT, name=f"cos{d}"))
                sin_sb.append(pw.tile([128, T], TABLE_DT, name=f"sin{d}"))
                ns_sb.append(pw.tile([128, T], TABLE_DT, name=f"ns{d}"))
                cpw_sb.append(pw.tile([128, T], TABLE_DT, name=f"cpw{d}"))
                spw_sb.append(pw.tile([128, T], TABLE_DT, name=f"spw{d}"))
            dd_sb = pw.tile([128, H], MM_DT, name="ddiag")
            fre_sb = pw.tile([128, S * S * 128], TABLE_DT, name="fretb")
            fim_sb = pw.tile([128, S * S * 128], TABLE_DT, name="fimtb")
            rcol_sb = pw.tile([128, 8], f32, name="rcol")
            eg_all = pw.tile([128, S * 128], TABLE_DT, name="egall")

            def emit_deferred_tables():
                # Act queue is needed for the first pb copies -- keep ALL
                # table loads off it.  gpsimd: B weights (PE-critical);
                # sync: per-slice tables in need order, then late tables.
                nc.gpsimd.dma_start(rcol_sb[:], rcol_d)
                for d in range(NSL):
                    nc.gpsimd.dma_start(brt_sb[d][:], brt_d[d * 128:(d + 1) * 128, 0:H // 2])
                    nc.sync.dma_start(bit_sb[d][:], bit_d[d * 128:(d + 1) * 128, 0:H // 2])
                for d in range(2):
                    nc.scalar.dma_start(cos_sb[d][:], cos_d[d * 128:(d + 1) * 128, :])
                    nc.scalar.dma_start(sin_sb[d][:], sin_d[d * 128:(d + 1) * 128, :])
                    nc.scalar.dma_start(ns_sb[d][:], ns_d[d * 128:(d + 1) * 128, :])
                for d in range(NSL):
                    nc.scalar.dma_start(brtB_sb[d][:], brt_d[d * 128:(d + 1) * 128, H // 2:H])
                    nc.sync.dma_start(bitB_sb[d][:], bit_d[d * 128:(d + 1) * 128, H // 2:H])
                for d in range(2, NSL):
                    nc.sync.dma_start(cos_sb[d][:], cos_d[d * 128:(d + 1) * 128, :])
                    nc.sync.dma_start(sin_sb[d][:], sin_d[d * 128:(d + 1) * 128, :])
                    nc.sync.dma_start(ns_sb[d][:], ns_d[d * 128:(d + 1) * 128, :])
                for d in range(NSL):
                    nc.sync.dma_start(cpw_sb[d][:], cpw_d[d * 128:(d + 1) * 128, :])
                    nc.sync.dma_start(spw_sb[d][:], spw_d[d * 128:(d + 1) * 128, :])
                    nc.sync.dma_start(ct_sb[d][:], ct_d[d * 128:(d + 1) * 128, :])
                nc.gpsimd.dma_start(dd_sb[:], dd_d)
                nc.gpsimd.dma_start(fre_sb[:], fre_d)
                nc.gpsimd.dma_start(fim_sb[:], fim_d)
                nc.vector.memzero(eg_all[:])

            saved = {}
            xt_tiles = {}

            def emit_xt_load(s):
                xt_sb = []
                for d in range(NSL):
                    t_ = px.tile([128, T], MM_DT, name=f"xt_s{s}_d{d}",
                                 tag="xt", bufs=24)
                    nc.sync.dma_start(t_[:], xt_d[s, d * 128:(d + 1) * 128, :])
                    xt_sb.append(t_)
                xt_tiles[s] = xt_sb

            def emit_scan_slice(s, sl, d_t, W_t):
                xt_sb = xt_tiles[s]
                if sl < 4:
                    brt_h = [brt_sb[d][:, sl * 128:(sl + 1) * 128] for d in range(NSL)]
                    bit_h = [bit_sb[d][:, sl * 128:(sl + 1) * 128] for d in range(NSL)]
                else:
                    brt_h = [brtB_sb[d][:, (sl - 4) * 128:(sl - 3) * 128] for d in range(NSL)]
                    bit_h = [bitB_sb[d][:, (sl - 4) * 128:(sl - 3) * 128] for d in range(NSL)]
                ps_re = pp.tile([128, T], f32, name=f"psre{s}_{sl}",
                                tag="bure", bufs=3)
                ps_im = pp.tile([128, T], f32, name=f"psim{s}_{sl}",
                                tag="buim", bufs=3)
                for d in range(NSL):
                    nc.tensor.matmul(ps_re[:], brt_h[d], xt_sb[d][:],
                                     start=(d == 0), stop=(d == NSL - 1))
                for d in range(NSL):
                    nc.tensor.matmul(ps_im[:], bit_h[d], xt_sb[d][:],
                                     start=(d == 0), stop=(d == NSL - 1))
                # packed [re|im] pipeline: pb = [Bu_re|Bu_im] (f16).
                # cts = [cos|cos]; direct terms in one packed DVE op, cross
                # terms on GpSimd: t13 = [sin.bim | -sin.bre]
                pb = pg.tile([128, 2 * T], f16, name=f"pb_{s}_{sl}", tag="pb", bufs=8)
                nc.scalar.copy(pb[:, 0:T], ps_re[:])
                nc.scalar.copy(pb[:, T:2 * T], ps_im[:])
                t02 = pg.tile([128, 2 * T], f16, name=f"t02_{s}_{sl}", tag="t02", bufs=3)
                cos_bc = cos_sb[sl][:].unsqueeze(1).broadcast_to([128, 2, T])
                nc.vector.tensor_tensor(
                    t02[:].rearrange("p (two t) -> p two t", two=2),
                    cos_bc,
                    pb[:].rearrange("p (two t) -> p two t", two=2), AL.mult)
                t13 = pg.tile([128, 2 * T], f16, name=f"t13_{s}_{sl}", tag="t13", bufs=3)
                nc.vector.tensor_tensor(t13[:, 0:T], sin_sb[sl][:], pb[:, T:2 * T], AL.mult)
                nc.vector.tensor_tensor(t13[:, T:2 * T], ns_sb[sl][:], pb[:, 0:T], AL.mult)
                gboth = pg.tile([128, 2 * T], f16, name=f"gb_{s}_{sl}", tag="gb", bufs=3)
                nc.vector.tensor_add(gboth[:], t02[:], t13[:])
                wboth = pg.tile([128, 2 * T], f16, name=f"wb_{s}_{sl}", tag="wb", bufs=4)
                rdec = rcol_sb[:, sl:sl + 1].broadcast_to([128, T])
                nc.vector.tensor_tensor_scan(wboth[:, 0:T], rdec, gboth[:, 0:T],
                                             0.0, AL.mult, AL.add)
                nc.vector.tensor_tensor_scan(wboth[:, T:2 * T], rdec, gboth[:, T:2 * T],
                                             0.0, AL.mult, AL.add)
                # local (carry-free) post-rotation on GpSimd:
                #   dloc = cos.Wre - sin.Wim
                p0 = pg.tile([128, T], f16, name=f"p0_{s}_{sl}", tag="p0", bufs=3)
                p1 = pg.tile([128, T], f16, name=f"p1_{s}_{sl}", tag="p1", bufs=3)
                nc.vector.tensor_tensor(p0[:], cos_sb[sl][:], wboth[:, 0:T], AL.mult)
                nc.vector.tensor_tensor(p1[:], sin_sb[sl][:], wboth[:, T:2 * T], AL.mult)
                dloc = pg.tile([128, T], f16, name=f"dl_{s}_{sl}", tag="dl", bufs=15)
                nc.vector.tensor_sub(dloc[:], p0[:], p1[:])
                d_t.append(dloc)
                W_t.append(wboth)

            def emit_publish(s, d_t, W_t):
                # publish raw W-last columns [128,16] f16 (re/im interleaved);
                # all constant factors (ROTT1, ROT1, Q powers) are folded into
                # the consume-side fre/fim tables on the host.
                wl16 = pcp.tile([128, 16], f16, name=f"wl{s}", tag="wl", bufs=2)
                for sl in range(NSL):
                    nc.scalar.copy(wl16[:, 2 * sl:2 * sl + 2],
                                   W_t[sl][:, T - 1:2 * T:T])
                pub_dr = pd.tile([128, 16], f16, name=f"pubdr{s}", tag="pubd", bufs=2)
                nc.gpsimd.dma_start(pub_dr[:], wl16[:])
                gat_dr = pd.tile([NC * 128, 16], f16, name=f"gatdr{s}", tag="gatd",
                                 bufs=2, addr_space="Shared")
                nc.gpsimd.collective_compute(
                    "AllGather", AL.bypass,
                    replica_groups=[list(range(NC))],
                    ins=[pub_dr[:].opt()],
                    outs=[gat_dr[:].opt()],
                )
                saved[s] = dict(xt_sb=xt_tiles[s], d_t=d_t, gat_dr=gat_dr)

            def emit_consume_head(s, hybrid=False):
                sv = saved[s]
                gv = sv["gat_dr"][:].rearrange("(c p) j -> p c j", c=NC)
                dst = eg_all[:, 128 * s:128 * (s + 1)].rearrange(
                    "p (c j) -> p c j", c=NC)
                nc.gpsimd.dma_start(dst, gv)

                def wsum(fold_sb, nmt, nmr, nm):
                    tmp = pcp.tile([128, S * 128], f16, name=f"{nmt}{s}",
                                   tag=nmt, bufs=1)
                    nc.vector.tensor_tensor(
                        tmp[:], fold_sb[:, 512 * s:512 * (s + 1)], eg_all[:],
                        AL.mult)
                    red1 = pcp.tile([128, 16], f32, name=f"{nmr}{s}", tag=nmr, bufs=2)
                    nc.vector.tensor_reduce(
                        red1[:].unsqueeze(2),
                        tmp[:].rearrange("p (g x) -> p x g", x=16),
                        AX.X, AL.add)
                    out = pcp.tile([128, 8], f32, name=f"{nm}{s}", tag=nm, bufs=2)
                    nc.vector.tensor_reduce(
                        out[:].unsqueeze(2),
                        red1[:].rearrange("p (sl pt) -> p sl pt", pt=2),
                        AX.X, AL.add)
                    return out

                v_re = wsum(fre_sb, "tta", "reda", "vre")
                v_im = wsum(fim_sb, "ttb", "redb", "vim")

                u_t = []
                for sl in range(NSL):
                    # carry correction in u-space; per-slice chains interleaved
                    # so u tiles emerge at engine cadence (psy matmuls chase
                    # them).  In the tail (hybrid), odd slices run entirely on
                    # the DVE via scalar_tensor_tensor so Act and DVE form two
                    # parallel pipes.
                    u = pg.tile([128, T], MM_DT, name=f"u{s}_{sl}", tag="u", bufs=10)
                    if hybrid and sl % 2 == 1:
                        st = pg.tile([128, T], f16, name=f"st{s}_{sl}", tag="stt", bufs=1)
                        nc.vector.scalar_tensor_tensor(
                            st[:], cpw_sb[sl][:], v_re[:, sl:sl + 1],
                            sv["d_t"][sl][:], AL.mult, AL.add)
                        nc.vector.scalar_tensor_tensor(
                            u[:], spw_sb[sl][:], v_im[:, sl:sl + 1],
                            st[:], AL.mult, AL.add)
                    else:
                        ucr = pg.tile([128, T], f16, name=f"ucr{s}_{sl}", tag="ucr", bufs=2)
                        uci = pg.tile([128, T], f16, name=f"uci{s}_{sl}", tag="uci", bufs=2)
                        nc.scalar.activation(ucr[:], cpw_sb[sl][:], AF.Copy,
                                             scale=v_re[:, sl:sl + 1])
                        nc.scalar.activation(uci[:], spw_sb[sl][:], AF.Copy,
                                             scale=v_im[:, sl:sl + 1])
                        scor = pg.tile([128, T], f16, name=f"scor{s}_{sl}", tag="scor", bufs=2)
                        nc.vector.tensor_add(scor[:], ucr[:], uci[:])
                        nc.vector.tensor_add(u[:], sv["d_t"][sl][:], scor[:])
                    u_t.append(u)
                sv["u_t"] = u_t

            def emit_consume_y(s):
                sv = saved.pop(s)
                xt_sb = sv["xt_sb"]
                u_t = sv["u_t"]
                for n in range(NSL):
                    ns = slice(n * 128, (n + 1) * 128)
                    psy = pp.tile([128, T], f32, name=f"psy{s}_{n}", tag="ytile", bufs=2)
                    nc.tensor.matmul(psy[:], dd_sb[:, ns], xt_sb[n][:],
                                     start=True, stop=False)
                    for sl in range(NSL):
                        nc.tensor.matmul(psy[:], ct_sb[sl][:, ns], u_t[sl][:],
                                         start=False, stop=(sl == NSL - 1))
                    ye = pg.tile([128, T], f16, name=f"ye{s}_{n}", tag="ye", bufs=2)
                    nc.scalar.copy(ye[:], psy[:])
                    nc.sync.dma_start(y_d[s, ns, :], ye[:])

            # ---- master schedule ----
            emit_xt_load(0)
            emit_deferred_tables()
            emit_xt_load(1)
            for s in (0, 1):
                d_t = []
                W_t = []
                for sl in range(NSL):
                    emit_scan_slice(s, sl, d_t, W_t)
                emit_publish(s, d_t, W_t)
                if s == 0 and S > 2:
                    emit_xt_load(2)
            for s in range(2, S):
                d_t = []
                W_t = []
                emit_scan_slice(s, 0, d_t, W_t)
                if s + 1 < S:
                    emit_xt_load(s + 1)
                emit_scan_slice(s, 1, d_t, W_t)
                emit_consume_head(s - 2)
                emit_scan_slice(s, 2, d_t, W_t)
                emit_scan_slice(s, 3, d_t, W_t)
                emit_scan_slice(s, 4, d_t, W_t)
                emit_scan_slice(s, 5, d_t, W_t)
                emit_scan_slice(s, 6, d_t, W_t)
                emit_consume_y(s - 2)
                emit_scan_slice(s, 7, d_t, W_t)
                emit_publish(s, d_t, W_t)
            emit_consume_head(S - 2, hybrid=True)
            emit_consume_head(S - 1, hybrid=True)
            emit_consume_y(S - 2)
            emit_consume_y(S - 1)

    nc.compile()
    _BUILD_CACHE["nc"] = nc
    return nc


def _prep(inputs, A_re, A_im, B_re, B_im, C, D):
    x = np.asarray(inputs, dtype=np.float32)
    A_re = np.asarray(A_re, dtype=np.float32)
    A_im = np.asarray(A_im, dtype=np.float32)
    B_re = np.asarray(B_re, dtype=np.float32)
    B_im = np.asarray(B_im, dtype=np.float32)
    C = np.asarray(C, dtype=np.float32)
    D = np.asarray(D, dtype=np.float32)
    A = A_re.astype(np.float64) + 1j * A_im.astype(np.float64)
    r = np.abs(A)
    th = np.angle(A)
    k = np.arange(T)
    COS = np.cos(th[:, None] * k)
    SIN = np.sin(th[:, None] * k)
    RPOW = r[:, None] ** (k + 1)
    CPW = COS * RPOW
    SPW = -SIN * RPOW
    Q = A ** T

    np16 = np.float16
    tb16 = np.float16

    brt = np.ascontiguousarray(B_re.T).astype(np16)
    bit = np.ascontiguousarray(B_im.T).astype(np16)
    ct = np.ascontiguousarray(C.T).astype(np16)
    cos_t = COS.astype(tb16)
    sin_t = SIN.astype(tb16)
    ns_t = (-SIN).astype(tb16)
    cpw_t = CPW.astype(tb16)
    spw_t = SPW.astype(tb16)

    # diag(D) stationaries: ddiag[p, n*128 + q] = D[n*128+p] if p==q else 0
    ddiag = np.zeros((128, H), np16)
    for n in range(NSL):
        np.fill_diagonal(ddiag[:, n * 128:(n + 1) * 128],
                         D[n * 128:(n + 1) * 128].astype(np16))

    # rcol[p, sl] = r[128*sl + p]  (per-slice scan decay columns)
    rcol = np.ascontiguousarray(r.reshape(NSL, 128).T).astype(np.float32)

    xT = np.ascontiguousarray(x.T)  # [M, L]

    # e^{i th T} = phase part of the per-chunk propagation; folds ROT1*ROTT1
    phT = np.exp(1j * th * T)

    in_maps = []
    for c in range(NC):
        # Direct-form fold tables: for consume slot s, source block s',
        # distance d=s-s', source core j:
        #   weight G = e^{i th T} * Q^(8d + c - j - 1)
        #   (d=0 only j<c; d<0 impossible -> zeros)
        # layout col = 512*s + 128*s' + 16*j + 2*sl + pt,
        # row p -> h = 128*sl + p
        fre = np.zeros((128, S * 512), np.float64)
        fim = np.zeros((128, S * 512), np.float64)
        for s in range(S):
            for sp in range(s + 1):
                dist = s - sp
                for j in range(NC):
                    if dist == 0 and j >= c:
                        continue
                    e = 8 * dist + c - j - 1
                    w = phT * Q ** e
                    wr = w.real.reshape(NSL, 128)
                    wi = w.imag.reshape(NSL, 128)
                    for sl in range(NSL):
                        col = 512 * s + 128 * sp + 16 * j + 2 * sl
                        fre[:, col] = wr[sl]
                        fre[:, col + 1] = -wi[sl]
                        fim[:, col] = wi[sl]
                        fim[:, col + 1] = wr[sl]
        fre16 = fre.astype(tb16)
        fim16 = fim.astype(tb16)

        xt = np.zeros((S, M, T), np16)
        for s in range(S):
            m = 8 * s + c
            xt[s] = xT[:, m * T:(m + 1) * T]

        in_maps.append({
            "xt": xt, "brt": brt, "bit": bit, "ct": ct, "ddiag": ddiag,
            "costb": cos_t, "sintb": sin_t, "nstb": ns_t,
            "cpwtb": cpw_t, "spwtb": spw_t,
            "fretb": fre16, "fimtb": fim16, "rcol": rcol,
        })
    return in_maps


LAST_RESULTS = {}


def kernel(inputs, A_re, A_im, B_re, B_im, C, D):
    nc = _build()
    in_maps = _prep(inputs, A_re, A_im, B_re, B_im, C, D)
    trace = os.environ.get("DIAG_TRACE", "") not in ("", "0")
    res = run_bass_kernel_spmd(nc, in_maps, core_ids=list(range(NC)),
                               trace=trace)
    LAST_RESULTS["exec_time_ns"] = res.exec_time_ns
    LAST_RESULTS["mean_exec_time_ns"] = res.mean_exec_time_ns
    yT = np.zeros((M, L), np.float32)
    for c in range(NC):
        yc = res.results[c]["y"].astype(np.float32)
        for s in range(S):
            m = 8 * s + c
            yT[:, m * T:(m + 1) * T] = yc[s]
    return np.ascontiguousarray(yT.T)
